# revision 50
# baseline (speedup 1.0000x reference)
"""MultiHeadAttention forward on 8 Trainium2 NeuronCores.

Sharding (Megatron-style tensor parallel x data parallel):
  core c (0..7): batch b = c // 4, head group g = c % 4 (4 of 16 heads).
  Wq/Wk/Wv column-sharded ([1024, 256] per core), Wo row-sharded
  ([256, 1024] per core). Each core computes a partial output
  [S, D] = attn(heads g) @ Wo_rows; the host sums the 4 partials per
  batch and adds bo (the "all-reduce" runs on host since full outputs
  are gathered anyway).

Device dataflow per core (all matmuls in float32r, full PE rate):
  QT/KT [dk-on-partition, S] via lhsT=W chunks, rhs=x^T chunks
  V natural [S-on-partition, 256] (+bias via ones-row matmul,
    +ones column appended for the softmax denominator)
  scores^T[k, q] per (head, q-block): lhsT=KT chunk, rhs=QT block
  E = exp(scores^T / sqrt(S)) via ACT, PSUM->SBUF
  attn^T [65, q] = accumulate lhsT=[V_h|1], rhs=E chunks
    (row 64 = softmax denominator; normalize by its reciprocal)
  O [q, 1024] = lhsT=attn^T chunks, rhs=Wo chunks; DMA PSUM->DRAM
"""

import math

import numpy as np

import concourse.bass as bass
import concourse.mybir as mybir
import concourse.tile as tile
from concourse import bacc
from concourse.bass_utils import run_bass_kernel_spmd

P = 128
B, S, D, H = 2, 2048, 1024, 16
NCORES = 8
GROUPS = NCORES // B          # 4 head-groups
HPC = H // GROUPS             # 4 heads per core
DK = D // H                   # 64
CPC = HPC * DK                # 256 cols per core
NP = CPC // P                 # 2 head pairs per core
DC = D // P                   # 8 contraction chunks over D
QB = 512                      # q block (matmul moving free dim)

F32 = mybir.dt.float32
F32R = mybir.dt.float32r

# DVE rejects partition-step-0 reads, so the normalize uses a
# matmul-based partition broadcast of the reciprocal row.
USE_PARTITION_BCAST = False


def build_program(seq=S):
    KT = seq // P             # k tiles
    NJ = seq // QB            # q blocks
    inv_sqrt_s = 1.0 / math.sqrt(S)  # reference scales by sqrt(full S) always

    nc = bacc.Bacc("TRN2", target_bir_lowering=False, debug=False,
                   num_devices=NCORES)
    xqT = nc.declare_dram_parameter("xqT", [D, seq], F32R, isOutput=False)
    xkT = nc.declare_dram_parameter("xkT", [D, seq], F32R, isOutput=False)
    xvT = nc.declare_dram_parameter("xvT", [D, seq], F32R, isOutput=False)
    wq = nc.declare_dram_parameter("wq", [D, CPC], F32R, isOutput=False)
    wk = nc.declare_dram_parameter("wk", [D, CPC], F32R, isOutput=False)
    wv = nc.declare_dram_parameter("wv", [D, CPC], F32R, isOutput=False)
    wo = nc.declare_dram_parameter("wo", [CPC, D], F32R, isOutput=False)
    bq = nc.declare_dram_parameter("bq", [CPC], F32, isOutput=False)
    bk = nc.declare_dram_parameter("bk", [CPC], F32, isOutput=False)
    bv = nc.declare_dram_parameter("bv", [1, CPC], F32R, isOutput=False)
    # float32r constants (memset can't write f32r: walrus ISA check)
    ones_row = nc.declare_dram_parameter("ones_row", [1, P], F32R,
                                         isOutput=False)
    vones = nc.declare_dram_parameter("vones", [P, KT * HPC], F32R,
                                      isOutput=False)
    out = nc.declare_dram_parameter("out", [seq, D], F32, isOutput=True)

    xqT_r = xqT.rearrange("(dc p) s -> p dc s", p=P)
    xkT_r = xkT.rearrange("(dc p) s -> p dc s", p=P)
    xvT_r = xvT.rearrange("(dc p) s -> p dc s", p=P)

    with tile.TileContext(nc) as tc:
        with tc.tile_pool(name="consts", bufs=1) as consts:
            bq_sb = consts.tile([P, NP], F32)
            bk_sb = consts.tile([P, NP], F32)
            bv_sb = consts.tile([1, CPC], F32R)
            ones_sb = consts.tile([1, P], F32R)
            # weight DMAs are emitted in consumption order (wk with stage-1
            # K, wq with Q, wv before V, wo before the attention pipeline)
            # so the x chunk DMAs are not queued behind cold weights.
            nc.sync.dma_start(bk_sb[:], bk.rearrange("(j p) -> p j", p=P))
            nc.sync.dma_start(bq_sb[:], bq.rearrange("(j p) -> p j", p=P))

            # Persistent activations. Per-pair / per-block tiles keep Tile's
            # dependency tracking fine-grained (stage overlap).
            qt_p = [consts.tile([P, seq], F32R, name=f"qt_p{j}")
                    for j in range(NP)]
            kt_p = [consts.tile([P, seq], F32R, name=f"kt_p{j}")
                    for j in range(NP)]
            v_sb = consts.tile([P, KT, HPC, DK + 1], F32R)
            at_j = [consts.tile([P, NP, QB], F32R, name=f"at_j{j}")
                    for j in range(NJ)]

            # ---- Stage 1 ----
            def emit_proj(name, x_r, w_src, b_sb, dst):
                with (
                    tc.tile_pool(name=f"xc_{name}", bufs=6) as xp,
                    tc.tile_pool(name=f"wp_{name}", bufs=1) as wp,
                    tc.tile_pool(name=f"ps_{name}", bufs=2 * NJ,
                                 space="PSUM") as psp,
                ):
                    # per-dc weight tiles, each DMA'd right after its x
                    # chunk: the dc=0 matmuls start after ~1.1MB instead of
                    # waiting for the whole weight matrix.
                    w_r = w_src.rearrange("(dc p) c -> p dc c", p=P)
                    w_dc = [wp.tile([P, CPC], F32R, tag=f"w{dc}",
                                    name=f"w_{name}{dc}")
                            for dc in range(DC)]
                    psq = [psp.tile([P, QB], F32, tag="qk", name=f"psq_{i}")
                           for i in range(NP * NJ)]
                    for dc in range(DC):
                        xt = xp.tile([P, seq], F32R, tag="xc")
                        nc.sync.dma_start(xt[:], x_r[:, dc])
                        nc.sync.dma_start(w_dc[dc][:], w_r[:, dc])
                        for j in range(NP):
                            for qc in range(NJ):
                                nc.tensor.matmul(
                                    psq[j * NJ + qc][:],
                                    w_dc[dc][:, j * P:(j + 1) * P],
                                    xt[:, qc * QB:(qc + 1) * QB],
                                    start=(dc == 0), stop=(dc == DC - 1),
                                )
                    for j in range(NP):
                        for qc in range(NJ):
                            # bias-add psum drains on ACT (idle in stage 1)
                            nc.scalar.activation(
                                dst[j][:, qc * QB:(qc + 1) * QB],
                                psq[j * NJ + qc][:],
                                mybir.ActivationFunctionType.Identity,
                                bias=b_sb[:, j:j + 1],
                            )

            # V projection, streamed like K/Q (dc-outer over 16 kt psums,
            # two kt sharing each psum bank) so PE tracks the x_v DMAs.
            # Emitted as a function so it can be placed AFTER the first
            # lookahead scores blocks: the in-order PE queue then has exp
            # feedstock issued ahead of the xv-DMA-paced V matmuls, and ACT
            # works through early softmax blocks during the x_v stream.
            def emit_v_block():
                with (
                    tc.tile_pool(name="xc_v", bufs=6) as xvp,
                    tc.tile_pool(name="wp_v", bufs=1) as wvp,
                    tc.tile_pool(name="ps_v", bufs=KT // 2,
                                 space="PSUM") as psvp,
                ):
                    wv_r = wv.rearrange("(dc p) c -> p dc c", p=P)
                    wv_dc = [wvp.tile([P, CPC], F32R, tag=f"w{dc}",
                                      name=f"w_v{dc}")
                             for dc in range(DC)]
                    psv = [psvp.tile([P, 2, CPC], F32, tag="v",
                                     name=f"psv_{k2}")
                           for k2 in range(KT // 2)]
                    # two kt share a psum bank => one accumulation group per
                    # bank: start only zeroes on the very first write, stop
                    # on the last bias matmul of the pair.
                    for dc in range(DC):
                        xt = xvp.tile([P, seq], F32R, tag="xc")
                        nc.sync.dma_start(xt[:], xvT_r[:, dc])
                        nc.sync.dma_start(wv_dc[dc][:], wv_r[:, dc])
                        for kt in range(KT):
                            nc.tensor.matmul(
                                psv[kt // 2][:, kt % 2],
                                xt[:, kt * P:(kt + 1) * P],
                                wv_dc[dc][:, :],
                                start=(dc == 0 and kt % 2 == 0), stop=False,
                            )
                        if dc == 0:
                            # constants are only needed at the bias/drain
                            # step; emitting them after xv0/wv0 keeps V's
                            # first matmuls off the DMA critical path.
                            nc.sync.dma_start(bv_sb[:], bv[:])
                            nc.sync.dma_start(ones_sb[:], ones_row[:])
                            with nc.allow_non_contiguous_dma(
                                    reason="one-time 32KB ones-column init"):
                                nc.sync.dma_start(
                                    v_sb[:, :, :, DK:DK + 1],
                                    vones.rearrange(
                                        "p (kt h) -> p kt h",
                                        kt=KT, h=HPC)[:, :, :, None],
                                )
                    for k2 in range(KT // 2):
                        for half in range(2):
                            nc.tensor.matmul(  # += ones^T @ bv  (bias add)
                                psv[k2][:, half], ones_sb[:], bv_sb[:],
                                start=False, stop=(half == 1),
                            )
                        for half in range(2):
                            # one strided copy per k-tile: [128, 4, 64] dest
                            # (skipping the ones column)
                            nc.vector.tensor_copy(
                                v_sb[:, 2 * k2 + half, :, 0:DK],
                                psv[k2][:, half].rearrange(
                                    "p (h d) -> p h d", h=HPC),
                            )

            # Emission order K -> V -> Q: V's xv-DMA-paced matmuls fill the
            # PE while x_q is still streaming, and V is complete before the
            # first attnV -- no V-wall inside the attention pipeline.
            emit_proj("k", xkT_r, wk, bk_sb, kt_p)
            emit_v_block()
            emit_proj("q", xqT_r, wq, bq_sb, qt_p)

            # ---- Stages 2+3: pipelined attention + output projection ----
            # Per (j, h) block: scores^T + exp; attnV trails LOOKAHEAD
            # blocks. O projection for q-block j is emitted right after its
            # last head's attnV.
            K2 = KT // 2  # two score k-tiles share one psum / exp op
            LOOKAHEAD = 2
            blocks = [(j, h) for j in range(NJ) for h in range(HPC)]

            def make_emit_scores(pool):
                def emit_scores(j, h, e2):
                    hp, hj = h % 2, h // 2
                    prow = slice(hp * DK, (hp + 1) * DK)
                    for k2 in range(K2):
                        pss = pool.tile([P, 2 * QB], F32, tag="s",
                                        name=f"pss_{j}_{h}_{k2}")
                        for half in range(2):
                            kt = 2 * k2 + half
                            nc.tensor.matmul(
                                pss[:, half * QB:(half + 1) * QB],
                                kt_p[hj][prow, kt * P:(kt + 1) * P],
                                qt_p[hj][prow, j * QB:(j + 1) * QB],
                                start=True, stop=True,
                            )
                        nc.scalar.activation(
                            e2[:, k2], pss[:],
                            mybir.ActivationFunctionType.Exp,
                            scale=inv_sqrt_s,
                        )
                return emit_scores

            with tc.tile_pool(name="epool", bufs=3) as ep:
                pending = []
                with (
                    tc.tile_pool(name="rpool", bufs=4) as rp,
                    tc.tile_pool(name="opool", bufs=4) as op,
                    tc.tile_pool(name="wop", bufs=1) as wop,
                    tc.tile_pool(name="ps_s", bufs=2, space="PSUM") as pss_p,
                    tc.tile_pool(name="ps_a", bufs=1, space="PSUM") as psa_p,
                    tc.tile_pool(name="ps_p", bufs=1, space="PSUM") as psp_p,
                    tc.tile_pool(name="ps_o", bufs=2, space="PSUM") as pso_p,
                ):
                    wo_sb = wop.tile([P, NP, D], F32R)
                    nc.sync.dma_start(wo_sb[:],
                                      wo.rearrange("(dj p) n -> p dj n", p=P))
                    emit_scores = make_emit_scores(pss_p)

                    def emit_attnv(j, h, e2):
                        hp, hj = h % 2, h // 2
                        prow = slice(hp * DK, (hp + 1) * DK)
                        psa = psa_p.tile([P, QB], F32, tag="a",
                                         name=f"psa_{j}_{h}")
                        for kt in range(KT):
                            nc.tensor.matmul(
                                psa[:DK + 1],
                                v_sb[:, kt, h, :],
                                e2[:, kt // 2,
                                   (kt % 2) * QB:(kt % 2 + 1) * QB],
                                start=(kt == 0), stop=(kt == KT - 1),
                            )
                        rc = rp.tile([1, QB], F32R, tag="rc")
                        with nc.allow_low_precision(
                                reason="f32r reciprocal for matmul bcast"):
                            nc.vector.reciprocal(rc[:], psa[DK:DK + 1, :])
                        prc = psp_p.tile([P, QB], F32, tag="p",
                                         name=f"prc_{j}_{h}")
                        nc.tensor.matmul(
                            prc[:DK], ones_sb[:, :DK], rc[:],
                            start=True, stop=True,
                        )
                        atmp = rp.tile([DK, QB], F32, tag="atmp")
                        nc.vector.tensor_copy(atmp[:], psa[:DK])
                        nc.vector.tensor_tensor(
                            at_j[j][prow, hj, :], atmp[:], prc[:DK],
                            mybir.AluOpType.mult,
                        )

                    def emit_oproj(j):
                        for ql in range(QB // P):
                            qt0 = j * (QB // P) + ql
                            o_sb = op.tile([P, D], F32, tag="o_sb")
                            for nh in range(D // QB):
                                pso = pso_p.tile([P, QB], F32, tag="o",
                                                 name=f"pso_{qt0}_{nh}")
                                for dj in range(NP):
                                    nc.tensor.matmul(
                                        pso[:],
                                        at_j[j][:, dj, ql * P:(ql + 1) * P],
                                        wo_sb[:, dj,
                                              nh * QB:(nh + 1) * QB],
                                        start=(dj == 0),
                                        stop=(dj == NP - 1),
                                    )
                                nc.vector.tensor_copy(
                                    o_sb[:, nh * QB:(nh + 1) * QB], pso[:])
                            # one merged 1MB output DMA per q-tile row
                            nc.sync.dma_start(
                                out[qt0 * P:(qt0 + 1) * P, :], o_sb[:],
                            )

                    for (j, h) in blocks:
                        e2 = ep.tile([P, K2, 2 * QB], F32R, tag="E",
                                     name=f"e2_{j}_{h}")
                        emit_scores(j, h, e2)
                        pending.append((j, h, e2))
                        if len(pending) > LOOKAHEAD:
                            jj, hh, ee = pending.pop(0)
                            emit_attnv(jj, hh, ee)
                            if hh == HPC - 1:
                                emit_oproj(jj)
                    for jj, hh, ee in pending:
                        emit_attnv(jj, hh, ee)
                        if hh == HPC - 1:
                            emit_oproj(jj)

    nc.compile()
    return nc


_PROGRAM_CACHE = {}


def _get_program(seq=S):
    if seq not in _PROGRAM_CACHE:
        _PROGRAM_CACHE[seq] = build_program(seq)
    return _PROGRAM_CACHE[seq]


def make_in_maps(queries, keys, values, Wq, bq, Wk, bk, Wv, bv, Wo, bo):
    """Per-core input dicts implementing the sharding."""
    f32 = np.float32
    seq = np.asarray(queries).shape[1]
    xT = {}
    for b in range(B):
        xT[b] = (
            np.ascontiguousarray(np.asarray(queries[b], dtype=f32).T),
            np.ascontiguousarray(np.asarray(keys[b], dtype=f32).T),
            np.ascontiguousarray(np.asarray(values[b], dtype=f32).T),
        )
    Wq, Wk, Wv, Wo = (np.asarray(a, dtype=f32) for a in (Wq, Wk, Wv, Wo))
    bq, bk, bv = (np.asarray(a, dtype=f32) for a in (bq, bk, bv))
    in_maps = []
    for c in range(NCORES):
        b, g = divmod(c, GROUPS)
        cs = slice(g * CPC, (g + 1) * CPC)
        qT, kT, vT = xT[b]
        in_maps.append({
            "xqT": qT, "xkT": kT, "xvT": vT,
            "wq": np.ascontiguousarray(Wq[:, cs]),
            "wk": np.ascontiguousarray(Wk[:, cs]),
            "wv": np.ascontiguousarray(Wv[:, cs]),
            "wo": np.ascontiguousarray(Wo[cs, :]),
            "bq": np.ascontiguousarray(bq[cs]),
            "bk": np.ascontiguousarray(bk[cs]),
            "bv": np.ascontiguousarray(bv[cs])[None, :],
            "ones_row": np.ones((1, P), dtype=f32),
            "vones": np.ones((P, (seq // P) * HPC), dtype=f32),
        })
    return in_maps


def combine_outputs(results, bo):
    """Host all-reduce of the Wo row-shard partials + bias."""
    bo = np.asarray(bo, dtype=np.float32)
    outs = []
    for b in range(B):
        acc = results[b * GROUPS]["out"].astype(np.float32).copy()
        for g in range(1, GROUPS):
            acc += results[b * GROUPS + g]["out"]
        outs.append(acc + bo)
    return np.stack(outs)


def kernel(queries, keys, values, Wq, bq, Wk, bk, Wv, bv, Wo, bo):
    nc = _get_program()
    in_maps = make_in_maps(queries, keys, values, Wq, bq, Wk, bk, Wv, bv,
                           Wo, bo)
    res = run_bass_kernel_spmd(nc, in_maps, list(range(NCORES)))
    return combine_outputs(res.results, bo)


# revision 53
# speedup vs baseline: 1.0059x; 1.0059x over previous
"""MultiHeadAttention forward on 8 Trainium2 NeuronCores.

Sharding (Megatron-style tensor parallel x data parallel):
  core c (0..7): batch b = c // 4, head group g = c % 4 (4 of 16 heads).
  Wq/Wk/Wv column-sharded ([1024, 256] per core), Wo row-sharded
  ([256, 1024] per core). Each core computes a partial output
  [S, D] = attn(heads g) @ Wo_rows; the host sums the 4 partials per
  batch and adds bo (the "all-reduce" runs on host since full outputs
  are gathered anyway).

Device dataflow per core (all matmuls in float32r, full PE rate):
  QT/KT [dk-on-partition, S] via lhsT=W chunks, rhs=x^T chunks
  V natural [S-on-partition, 256] (+bias via ones-row matmul,
    +ones column appended for the softmax denominator)
  scores^T[k, q] per (head, q-block): lhsT=KT chunk, rhs=QT block
  E = exp(scores^T / sqrt(S)) via ACT, PSUM->SBUF
  attn^T [65, q] = accumulate lhsT=[V_h|1], rhs=E chunks
    (row 64 = softmax denominator; normalize by its reciprocal)
  O [q, 1024] = lhsT=attn^T chunks, rhs=Wo chunks; DMA PSUM->DRAM
"""

import math

import numpy as np

import concourse.bass as bass
import concourse.mybir as mybir
import concourse.tile as tile
from concourse import bacc
from concourse.bass_utils import run_bass_kernel_spmd

P = 128
B, S, D, H = 2, 2048, 1024, 16
NCORES = 8
GROUPS = NCORES // B          # 4 head-groups
HPC = H // GROUPS             # 4 heads per core
DK = D // H                   # 64
CPC = HPC * DK                # 256 cols per core
NP = CPC // P                 # 2 head pairs per core
DC = D // P                   # 8 contraction chunks over D
QB = 512                      # q block (matmul moving free dim)

F32 = mybir.dt.float32
F32R = mybir.dt.float32r

# DVE rejects partition-step-0 reads, so the normalize uses a
# matmul-based partition broadcast of the reciprocal row.
USE_PARTITION_BCAST = False


def build_program(seq=S):
    KT = seq // P             # k tiles
    NJ = seq // QB            # q blocks
    inv_sqrt_s = 1.0 / math.sqrt(S)  # reference scales by sqrt(full S) always

    nc = bacc.Bacc("TRN2", target_bir_lowering=False, debug=False,
                   num_devices=NCORES)
    xqT = nc.declare_dram_parameter("xqT", [D, seq], F32R, isOutput=False)
    xkT = nc.declare_dram_parameter("xkT", [D, seq], F32R, isOutput=False)
    xvT = nc.declare_dram_parameter("xvT", [D, seq], F32R, isOutput=False)
    wq = nc.declare_dram_parameter("wq", [D, CPC], F32R, isOutput=False)
    wk = nc.declare_dram_parameter("wk", [D, CPC], F32R, isOutput=False)
    wv = nc.declare_dram_parameter("wv", [D, CPC], F32R, isOutput=False)
    wo = nc.declare_dram_parameter("wo", [CPC, D], F32R, isOutput=False)
    bq = nc.declare_dram_parameter("bq", [CPC], F32, isOutput=False)
    bk = nc.declare_dram_parameter("bk", [CPC], F32, isOutput=False)
    bv = nc.declare_dram_parameter("bv", [1, CPC], F32R, isOutput=False)
    # float32r constants (memset can't write f32r: walrus ISA check)
    ones_row = nc.declare_dram_parameter("ones_row", [1, P], F32R,
                                         isOutput=False)
    vones = nc.declare_dram_parameter("vones", [P, KT * HPC], F32R,
                                      isOutput=False)
    out = nc.declare_dram_parameter("out", [seq, D], F32, isOutput=True)

    xqT_r = xqT.rearrange("(dc p) s -> p dc s", p=P)
    xkT_r = xkT.rearrange("(dc p) s -> p dc s", p=P)
    xvT_r = xvT.rearrange("(dc p) s -> p dc s", p=P)

    with tile.TileContext(nc) as tc:
        with tc.tile_pool(name="consts", bufs=1) as consts:
            bq_sb = consts.tile([P, NP], F32)
            bk_sb = consts.tile([P, NP], F32)
            bv_sb = consts.tile([1, CPC], F32R)
            ones_sb = consts.tile([1, P], F32R)
            # weight DMAs are emitted in consumption order (wk with stage-1
            # K, wq with Q, wv before V, wo before the attention pipeline)
            # so the x chunk DMAs are not queued behind cold weights.
            nc.sync.dma_start(bk_sb[:], bk.rearrange("(j p) -> p j", p=P))
            nc.sync.dma_start(bq_sb[:], bq.rearrange("(j p) -> p j", p=P))

            # Persistent activations. Per-pair / per-block tiles keep Tile's
            # dependency tracking fine-grained (stage overlap).
            qt_t = [[consts.tile([P, QB], F32R, name=f"qt_{j}_{qc}")
                     for qc in range(NJ)] for j in range(NP)]
            kt_p = [consts.tile([P, seq], F32R, name=f"kt_p{j}")
                    for j in range(NP)]
            v_sb = consts.tile([P, KT, HPC, DK + 1], F32R)
            at_j = [consts.tile([P, NP, QB], F32R, name=f"at_j{j}")
                    for j in range(NJ)]

            # ---- Stage 1 ----
            def emit_proj(name, x_r, w_src, b_sb, dst_fn, dve_pairs=()):
                with (
                    tc.tile_pool(name=f"xc_{name}", bufs=6) as xp,
                    tc.tile_pool(name=f"wp_{name}", bufs=1) as wp,
                    tc.tile_pool(name=f"ps_{name}", bufs=2 * NJ,
                                 space="PSUM") as psp,
                ):
                    # per-dc weight tiles, each DMA'd right after its x
                    # chunk: the dc=0 matmuls start after ~1.1MB instead of
                    # waiting for the whole weight matrix.
                    w_r = w_src.rearrange("(dc p) c -> p dc c", p=P)
                    w_dc = [wp.tile([P, CPC], F32R, tag=f"w{dc}",
                                    name=f"w_{name}{dc}")
                            for dc in range(DC)]
                    psq = [psp.tile([P, QB], F32, tag="qk", name=f"psq_{i}")
                           for i in range(NP * NJ)]
                    for dc in range(DC):
                        xt = xp.tile([P, seq], F32R, tag="xc")
                        nc.sync.dma_start(xt[:], x_r[:, dc])
                        nc.sync.dma_start(w_dc[dc][:], w_r[:, dc])
                        for j in range(NP):
                            for qc in range(NJ):
                                nc.tensor.matmul(
                                    psq[j * NJ + qc][:],
                                    w_dc[dc][:, j * P:(j + 1) * P],
                                    xt[:, qc * QB:(qc + 1) * QB],
                                    start=(dc == 0), stop=(dc == DC - 1),
                                )
                    for j in range(NP):
                        for qc in range(NJ):
                            # bias-add psum drains: ACT (idle in stage 1)
                            # except pairs routed to DVE to keep the first
                            # exps from queueing behind them on ACT.
                            if j in dve_pairs:
                                nc.vector.tensor_scalar_add(
                                    dst_fn(j, qc),
                                    psq[j * NJ + qc][:],
                                    b_sb[:, j:j + 1],
                                )
                            else:
                                nc.scalar.activation(
                                    dst_fn(j, qc),
                                    psq[j * NJ + qc][:],
                                    mybir.ActivationFunctionType.Identity,
                                    bias=b_sb[:, j:j + 1],
                                )

            # V projection, streamed like K/Q (dc-outer over 16 kt psums,
            # two kt sharing each psum bank) so PE tracks the x_v DMAs.
            # Emitted as a function so it can be placed AFTER the first
            # lookahead scores blocks: the in-order PE queue then has exp
            # feedstock issued ahead of the xv-DMA-paced V matmuls, and ACT
            # works through early softmax blocks during the x_v stream.
            def emit_v_block():
                with (
                    tc.tile_pool(name="xc_v", bufs=6) as xvp,
                    tc.tile_pool(name="wp_v", bufs=1) as wvp,
                    tc.tile_pool(name="ps_v", bufs=KT // 2,
                                 space="PSUM") as psvp,
                ):
                    wv_r = wv.rearrange("(dc p) c -> p dc c", p=P)
                    wv_dc = [wvp.tile([P, CPC], F32R, tag=f"w{dc}",
                                      name=f"w_v{dc}")
                             for dc in range(DC)]
                    psv = [psvp.tile([P, 2, CPC], F32, tag="v",
                                     name=f"psv_{k2}")
                           for k2 in range(KT // 2)]
                    # two kt share a psum bank => one accumulation group per
                    # bank: start only zeroes on the very first write, stop
                    # on the last bias matmul of the pair.
                    for dc in range(DC):
                        xt = xvp.tile([P, seq], F32R, tag="xc")
                        nc.sync.dma_start(xt[:], xvT_r[:, dc])
                        nc.sync.dma_start(wv_dc[dc][:], wv_r[:, dc])
                        for kt in range(KT):
                            nc.tensor.matmul(
                                psv[kt // 2][:, kt % 2],
                                xt[:, kt * P:(kt + 1) * P],
                                wv_dc[dc][:, :],
                                start=(dc == 0 and kt % 2 == 0), stop=False,
                            )
                        if dc == 0:
                            # constants are only needed at the bias/drain
                            # step; emitting them after xv0/wv0 keeps V's
                            # first matmuls off the DMA critical path.
                            nc.sync.dma_start(bv_sb[:], bv[:])
                            nc.sync.dma_start(ones_sb[:], ones_row[:])
                            with nc.allow_non_contiguous_dma(
                                    reason="one-time 32KB ones-column init"):
                                nc.sync.dma_start(
                                    v_sb[:, :, :, DK:DK + 1],
                                    vones.rearrange(
                                        "p (kt h) -> p kt h",
                                        kt=KT, h=HPC)[:, :, :, None],
                                )
                    for k2 in range(KT // 2):
                        for half in range(2):
                            nc.tensor.matmul(  # += ones^T @ bv  (bias add)
                                psv[k2][:, half], ones_sb[:], bv_sb[:],
                                start=False, stop=(half == 1),
                            )
                        for half in range(2):
                            # one strided copy per k-tile: [128, 4, 64] dest
                            # (skipping the ones column)
                            nc.vector.tensor_copy(
                                v_sb[:, 2 * k2 + half, :, 0:DK],
                                psv[k2][:, half].rearrange(
                                    "p (h d) -> p h d", h=HPC),
                            )

            # Emission order K -> V -> Q: V's xv-DMA-paced matmuls fill the
            # PE while x_q is still streaming, and V is complete before the
            # first attnV -- no V-wall inside the attention pipeline.
            emit_proj("k", xkT_r, wk, bk_sb,
                      lambda j, qc: kt_p[j][:, qc * QB:(qc + 1) * QB])
            emit_v_block()
            emit_proj("q", xqT_r, wq, bq_sb,
                      lambda j, qc: qt_t[j][qc][:], dve_pairs=(1,))

            # ---- Stages 2+3: pipelined attention + output projection ----
            # Per (j, h) block: scores^T + exp; attnV trails LOOKAHEAD
            # blocks. O projection for q-block j is emitted right after its
            # last head's attnV.
            K2 = KT // 2  # two score k-tiles share one psum / exp op
            LOOKAHEAD = 2
            blocks = [(j, h) for j in range(NJ) for h in range(HPC)]

            def make_emit_scores(pool):
                def emit_scores(j, h, e2):
                    hp, hj = h % 2, h // 2
                    prow = slice(hp * DK, (hp + 1) * DK)
                    for k2 in range(K2):
                        pss = pool.tile([P, 2 * QB], F32, tag="s",
                                        name=f"pss_{j}_{h}_{k2}")
                        for half in range(2):
                            kt = 2 * k2 + half
                            nc.tensor.matmul(
                                pss[:, half * QB:(half + 1) * QB],
                                kt_p[hj][prow, kt * P:(kt + 1) * P],
                                qt_t[hj][j][prow, :],
                                start=True, stop=True,
                            )
                        nc.scalar.activation(
                            e2[:, k2], pss[:],
                            mybir.ActivationFunctionType.Exp,
                            scale=inv_sqrt_s,
                        )
                return emit_scores

            with tc.tile_pool(name="epool", bufs=3) as ep:
                pending = []
                with (
                    tc.tile_pool(name="rpool", bufs=4) as rp,
                    tc.tile_pool(name="opool", bufs=4) as op,
                    tc.tile_pool(name="wop", bufs=1) as wop,
                    tc.tile_pool(name="ps_s", bufs=2, space="PSUM") as pss_p,
                    tc.tile_pool(name="ps_a", bufs=1, space="PSUM") as psa_p,
                    tc.tile_pool(name="ps_p", bufs=1, space="PSUM") as psp_p,
                    tc.tile_pool(name="ps_o", bufs=2, space="PSUM") as pso_p,
                ):
                    wo_sb = wop.tile([P, NP, D], F32R)
                    nc.sync.dma_start(wo_sb[:],
                                      wo.rearrange("(dj p) n -> p dj n", p=P))
                    emit_scores = make_emit_scores(pss_p)

                    def emit_attnv(j, h, e2):
                        hp, hj = h % 2, h // 2
                        prow = slice(hp * DK, (hp + 1) * DK)
                        psa = psa_p.tile([P, QB], F32, tag="a",
                                         name=f"psa_{j}_{h}")
                        for kt in range(KT):
                            nc.tensor.matmul(
                                psa[:DK + 1],
                                v_sb[:, kt, h, :],
                                e2[:, kt // 2,
                                   (kt % 2) * QB:(kt % 2 + 1) * QB],
                                start=(kt == 0), stop=(kt == KT - 1),
                            )
                        rc = rp.tile([1, QB], F32R, tag="rc")
                        with nc.allow_low_precision(
                                reason="f32r reciprocal for matmul bcast"):
                            nc.vector.reciprocal(rc[:], psa[DK:DK + 1, :])
                        prc = psp_p.tile([P, QB], F32, tag="p",
                                         name=f"prc_{j}_{h}")
                        nc.tensor.matmul(
                            prc[:DK], ones_sb[:, :DK], rc[:],
                            start=True, stop=True,
                        )
                        atmp = rp.tile([DK, QB], F32, tag="atmp")
                        nc.vector.tensor_copy(atmp[:], psa[:DK])
                        nc.vector.tensor_tensor(
                            at_j[j][prow, hj, :], atmp[:], prc[:DK],
                            mybir.AluOpType.mult,
                        )

                    def emit_oproj(j):
                        for ql in range(QB // P):
                            qt0 = j * (QB // P) + ql
                            o_sb = op.tile([P, D], F32, tag="o_sb")
                            for nh in range(D // QB):
                                pso = pso_p.tile([P, QB], F32, tag="o",
                                                 name=f"pso_{qt0}_{nh}")
                                for dj in range(NP):
                                    nc.tensor.matmul(
                                        pso[:],
                                        at_j[j][:, dj, ql * P:(ql + 1) * P],
                                        wo_sb[:, dj,
                                              nh * QB:(nh + 1) * QB],
                                        start=(dj == 0),
                                        stop=(dj == NP - 1),
                                    )
                                nc.vector.tensor_copy(
                                    o_sb[:, nh * QB:(nh + 1) * QB], pso[:])
                            # one merged 1MB output DMA per q-tile row
                            nc.sync.dma_start(
                                out[qt0 * P:(qt0 + 1) * P, :], o_sb[:],
                            )

                    for (j, h) in blocks:
                        e2 = ep.tile([P, K2, 2 * QB], F32R, tag="E",
                                     name=f"e2_{j}_{h}")
                        emit_scores(j, h, e2)
                        pending.append((j, h, e2))
                        if len(pending) > LOOKAHEAD:
                            jj, hh, ee = pending.pop(0)
                            emit_attnv(jj, hh, ee)
                            if hh == HPC - 1:
                                emit_oproj(jj)
                    for jj, hh, ee in pending:
                        emit_attnv(jj, hh, ee)
                        if hh == HPC - 1:
                            emit_oproj(jj)

    nc.compile()
    return nc


_PROGRAM_CACHE = {}


def _get_program(seq=S):
    if seq not in _PROGRAM_CACHE:
        _PROGRAM_CACHE[seq] = build_program(seq)
    return _PROGRAM_CACHE[seq]


def make_in_maps(queries, keys, values, Wq, bq, Wk, bk, Wv, bv, Wo, bo):
    """Per-core input dicts implementing the sharding."""
    f32 = np.float32
    seq = np.asarray(queries).shape[1]
    xT = {}
    for b in range(B):
        xT[b] = (
            np.ascontiguousarray(np.asarray(queries[b], dtype=f32).T),
            np.ascontiguousarray(np.asarray(keys[b], dtype=f32).T),
            np.ascontiguousarray(np.asarray(values[b], dtype=f32).T),
        )
    Wq, Wk, Wv, Wo = (np.asarray(a, dtype=f32) for a in (Wq, Wk, Wv, Wo))
    bq, bk, bv = (np.asarray(a, dtype=f32) for a in (bq, bk, bv))
    in_maps = []
    for c in range(NCORES):
        b, g = divmod(c, GROUPS)
        cs = slice(g * CPC, (g + 1) * CPC)
        qT, kT, vT = xT[b]
        in_maps.append({
            "xqT": qT, "xkT": kT, "xvT": vT,
            "wq": np.ascontiguousarray(Wq[:, cs]),
            "wk": np.ascontiguousarray(Wk[:, cs]),
            "wv": np.ascontiguousarray(Wv[:, cs]),
            "wo": np.ascontiguousarray(Wo[cs, :]),
            "bq": np.ascontiguousarray(bq[cs]),
            "bk": np.ascontiguousarray(bk[cs]),
            "bv": np.ascontiguousarray(bv[cs])[None, :],
            "ones_row": np.ones((1, P), dtype=f32),
            "vones": np.ones((P, (seq // P) * HPC), dtype=f32),
        })
    return in_maps


def combine_outputs(results, bo):
    """Host all-reduce of the Wo row-shard partials + bias."""
    bo = np.asarray(bo, dtype=np.float32)
    outs = []
    for b in range(B):
        acc = results[b * GROUPS]["out"].astype(np.float32).copy()
        for g in range(1, GROUPS):
            acc += results[b * GROUPS + g]["out"]
        outs.append(acc + bo)
    return np.stack(outs)


def kernel(queries, keys, values, Wq, bq, Wk, bk, Wv, bv, Wo, bo):
    nc = _get_program()
    in_maps = make_in_maps(queries, keys, values, Wq, bq, Wk, bk, Wv, bv,
                           Wo, bo)
    res = run_bass_kernel_spmd(nc, in_maps, list(range(NCORES)))
    return combine_outputs(res.results, bo)
